# revision 1
# baseline (speedup 1.0000x reference)
"""Trainium2 Bass kernel for nn_DescriptionEmbedding (attention-pooling).

Math: for each feature f, attention over W hidden words:
  score[f,w] = sum_h u[h] * tanh(a[f,h] + c[w,h]),  a = fe@W1, c = he@W2 + b
  attn = softmax_w(masked exp), context[f] = sum_w attn*he[w], out = values@context

Key reformulation (exact identity + short series):
  tanh(a+c) = (ta+tc)/(1+ta*tc),  ta=tanh(a), tc=tanh(c)
            = ta + sum_{j>=1} (-1)^(j-1) (1-ta^2) ta^(j-1) * tc^j
The j=0 term (u.ta summed over h) is constant in w -> cancels in softmax -> dropped.
Truncated at j<=2 (validated: out rel err ~2e-5):
  S~[w,f] = tc[w,:]   @ (u*(1-ta^2))[f,:].T
          + tc2[w,:]  @ (-u*(1-ta^2)*ta)[f,:].T
i.e. ONE K=128 matmul per 125-row w-chunk producing scores directly in [w,f]
layout, which feeds the context matmul with no on-chip transposes.

Sharding: F=2000 split 8 x 250 (padded to 256 for full-rate fp32r matmuls);
each core computes its features' context and a partial [B,16] of the final
values@context over its feature shard; host sums the 8 partials.
"""
import os
import sys

import numpy as np

F, W, E, H, B = 2000, 4000, 16, 64, 256
NCORES = 8
FS = F // NCORES          # 250 features per core
FP = 256                  # padded feature columns (fp32r full rate needs N>=256)
PW = 125                  # w-chunk partition size (4000 = 32*125)
NWC = W // PW             # 32 w-chunks
NQ = 8                    # quads (4 w-chunks each) per core
F32 = None                # filled after concourse import


def _import_concourse():
    # bass2jax executes via jax PJRT on the neuron devices; a cpu platform
    # pin would hide them. Clear it if jax hasn't been imported yet.
    if "jax" not in sys.modules and os.environ.get("JAX_PLATFORMS") == "cpu":
        del os.environ["JAX_PLATFORMS"]
    try:
        import concourse.bass  # noqa: F401
    except ImportError:
        for p in ("/opt/trn_rl_repo", os.path.expanduser("~/trn_rl_repo")):
            if os.path.isdir(p) and p not in sys.path:
                sys.path.insert(0, p)
        import concourse.bass  # noqa: F401


def build_nc(reps=1):
    _import_concourse()
    import concourse.bass as bass
    import concourse.mybir as mybir
    import concourse.tile as tile
    from concourse import bacc
    from concourse.alu_op_type import AluOpType
    from concourse.masks import make_identity

    f32 = mybir.dt.float32
    f16 = mybir.dt.float16
    f32r = mybir.dt.float32r
    u8 = mybir.dt.uint8
    ACT = mybir.ActivationFunctionType

    nc = bacc.Bacc(None, target_bir_lowering=False, debug=False)

    # blob layout (f32r, [64, 386]): col 0 = bT, col 1 = uT,
    # [0:16, 2:66] = w1, [0:16, 66:130] = w2, [0:16, 130:386] = feT
    blob = nc.dram_tensor("blob", [H, 386], f32r, kind="ExternalInput")
    heT = nc.dram_tensor("heT", [E, W], f32r, kind="ExternalInput")
    heo = nc.dram_tensor("heo", [PW, NWC, 17], f32r, kind="ExternalInput")
    maskT = nc.dram_tensor("maskT", [2, PW, 16, FP], u8, kind="ExternalInput")
    vT = nc.dram_tensor("vT", [FP, B], f32, kind="ExternalInput")
    out = nc.dram_tensor("out", [B, E], f32, kind="ExternalOutput")

    r = lambda ap: ap if ap.dtype == f32r else ap.bitcast(f32r)

    import contextlib

    with tile.TileContext(nc) as tc:
        loop_cm = tc.For_i(0, reps, 1) if reps > 1 else contextlib.nullcontext()
        with (
            loop_cm,
            tc.tile_pool(name="consts", bufs=1) as consts,
            tc.tile_pool(name="prep_ps", bufs=2, space="PSUM") as prep_ps,
            tc.tile_pool(name="s_ps", bufs=2, space="PSUM") as s_ps,
            tc.tile_pool(name="ctx_ps", bufs=1, space="PSUM") as ctx_ps,
            tc.tile_pool(name="masks", bufs=2) as maskpool,
            tc.tile_pool(name="escore", bufs=4) as epool,
            tc.tile_pool(name="small", bufs=2) as small,
        ):
            # ---- constant loads -------------------------------------------
            blobs = consts.tile([H, 386], f32r)
            heTs = consts.tile([E, W], f32r)
            heos = consts.tile([PW, NWC, 17], f32r)
            vTs = consts.tile([128, 2, B], f32)
            ident = consts.tile([32, 32], f32)
            nc.sync.dma_start(blobs[:], blob[:])
            nc.sync.dma_start(heTs[:], heT[:])
            w1s = blobs[0:E, 2:66]
            w2s = blobs[0:E, 66:130]
            feTs = blobs[0:E, 130:386]
            bTs = blobs[:, 0:1].bitcast(f32)
            uTs = blobs[:, 1:2].bitcast(f32)
            make_identity(nc, ident[:])

            # ---- P-side blocks: PT[0:64]=u*(1-ta^2), PT[64:128]=-u*(1-ta^2)*ta
            pf = prep_ps.tile([H, FP], f32, tag="prep")
            nc.tensor.matmul(pf[:], w1s, feTs, start=True, stop=True)
            ta = small.tile([H, FP], f32)
            nc.scalar.activation(ta[:], pf[:], ACT.Tanh)
            PT = consts.tile([128, FP], f32r)
            tmp = small.tile([H, FP], f32)
            # tmp = 1 - ta^2
            nc.vector.tensor_tensor(tmp[:], ta[:], ta[:], AluOpType.mult)
            nc.vector.tensor_scalar(tmp[:], tmp[:], -1.0, 1.0,
                                    AluOpType.mult, AluOpType.add)
            # PT[0:64] = u * tmp
            nc.vector.tensor_scalar_mul(PT[0:H, :], tmp[:], uTs)
            # nta = -ta ; PT[64:128] = PT[0:64] * nta
            nta = small.tile([H, FP], f32)
            nc.vector.tensor_scalar_mul(nta[:], ta[:], -1.0)
            nc.vector.tensor_tensor(PT[H:128, :], PT[0:H, :], nta[:],
                                    AluOpType.mult)

            # ---- main structure: QT-tile prep interleaved with score quads --
            pctx = ctx_ps.tile([17, FP], f32)
            QTs = [consts.tile([128, 8 * PW], f32r, name=f"QT{t}", tag=f"qt{t}")
                   for t in range(4)]
            mqs = []
            for hq in range(2):
                mqh = maskpool.tile([PW, 16, FP], u8, name=f"mqh{hq}",
                                    tag="mqh")
                mqs.append(mqh)
            nc.sync.dma_start(mqs[0][:], maskT[0])
            nc.sync.dma_start(heos[:], heo[:])
            nc.sync.dma_start(mqs[1][:], maskT[1])
            nc.sync.dma_start(vTs[:], vT[:].rearrange("(q p) b -> p q b", p=128))

            def prep_tile(t):
                # QT[t] rows 0:64 = tc, rows 64:128 = tc^2
                hp = s_ps.tile([H, 2, 512], f32, tag="ps", name="hp")
                for half in range(2):
                    c = 2 * t + half
                    nc.tensor.matmul(hp[:, half, 0:500], w2s,
                                     heTs[:, c * 500:(c + 1) * 500],
                                     start=True, stop=True)
                nc.scalar.activation(
                    QTs[t][0:H, :].rearrange("p (i c) -> p i c", i=2),
                    hp[:, :, 0:500], ACT.Tanh, bias=bTs)
                nc.vector.tensor_tensor(QTs[t][H:128, :], QTs[t][0:H, :],
                                        QTs[t][0:H, :], AluOpType.mult)

            def emit_ctx(q, eq):
                for i in range(4):
                    wc = 4 * q + i
                    nc.tensor.matmul(pctx[:], r(heos[:, wc, :]), r(eq[:, i, :]),
                                     start=(wc == 0), stop=(wc == NWC - 1))

            state = {"eqs": []}

            def quad(q):
                mq = mqs[q // 4][:, (q % 4) * 4:(q % 4) * 4 + 4, :]
                ps = s_ps.tile([PW, 4, FP], f32, tag="ps", name="ps")
                for i in range(4):
                    wc = 4 * q + i
                    qt = QTs[wc // 8]
                    wsl = slice((wc % 8) * PW, (wc % 8) * PW + PW)
                    nc.tensor.matmul(ps[:, i, :], r(qt[:, wsl]), r(PT[:]),
                                     start=True, stop=True)
                eq = epool.tile([PW, 4, FP], f32r)
                nc.scalar.activation(eq[:], ps[:], ACT.Exp)
                nc.vector.tensor_tensor(eq[:], eq[:], mq, AluOpType.mult)
                state["eqs"].append(eq)
                if len(state["eqs"]) >= 3:
                    emit_ctx(q - 2, state["eqs"].pop(0))

            for t in range(4):
                prep_tile(t)
            for q in range(NQ):
                quad(q)
            for k, eq in enumerate(state["eqs"]):
                emit_ctx(NQ - len(state["eqs"]) + k, eq)

            # ---- epilogue: normalize context, partial values @ ctx ---------
            ctxT = small.tile([17, FP], f32, tag="ctxT")
            nc.scalar.activation(ctxT[:], pctx[:], ACT.Copy)
            ctxf = small.tile([128, 2, 17], f32, tag="ctxf")
            for h in range(2):
                pt = prep_ps.tile([128, 17], f32, tag="prep")
                nc.tensor.transpose(pt[:], ctxT[:, h * 128:(h + 1) * 128],
                                    ident[0:17, 0:17])
                nc.vector.tensor_copy(ctxf[:, h, :], pt[:])
            rv = small.tile([128, 2], f32, tag="rv")
            nc.vector.reciprocal(rv[:], ctxf[:, :, 16])
            ctxn = small.tile([128, 2, E], f32, tag="ctxn")
            for h in range(2):
                nc.vector.tensor_scalar_mul(ctxn[:, h, :], ctxf[:, h, 0:E],
                                            rv[:, h:h + 1])
            outsb = small.tile([128, 2, E], f32, tag="outsb")
            for bh in range(2):
                po = prep_ps.tile([128, E], f32, tag="prep")
                for h in range(2):
                    nc.tensor.matmul(po[:], vTs[:, h, bh * 128:(bh + 1) * 128],
                                     ctxn[:, h, :], start=(h == 0), stop=(h == 1))
                nc.vector.tensor_copy(outsb[:, bh, :], po[:])
            nc.sync.dma_start(out[:].rearrange("(h p) e -> p h e", p=128),
                              outsb[:])

    nc.compile()
    return nc


def shard_inputs(values, feature_emb, hidden_emb, W_w, b_w, W_u, mask):
    """Host-side shard/layout prep. Returns per-core input maps."""
    values = np.ascontiguousarray(values, dtype=np.float32)
    fe = np.ascontiguousarray(feature_emb, dtype=np.float32)
    he = np.ascontiguousarray(hidden_emb, dtype=np.float32)
    W_w = np.ascontiguousarray(W_w, dtype=np.float32)
    b_w = np.ascontiguousarray(b_w, dtype=np.float32)
    W_u = np.ascontiguousarray(W_u, dtype=np.float32)
    m = np.asarray(mask).reshape(F, W)

    heT = np.ascontiguousarray(he.T)                      # [E, W]
    heo_flat = np.concatenate([he, np.ones((W, 1), np.float32)], axis=1)  # [W, 17]
    # packed [PW, NWC, 17]: row w = n*PW + p  ->  [p, n, :]
    heo = np.ascontiguousarray(heo_flat.reshape(NWC, PW, 17).transpose(1, 0, 2))
    w1 = np.ascontiguousarray(W_w[:E])                    # [E, H]
    w2 = np.ascontiguousarray(W_w[E:])                    # [E, H]
    bT = np.ascontiguousarray(b_w.reshape(H, 1))
    uT = np.ascontiguousarray(W_u.reshape(H, 1))
    feT_full = fe.T                                       # [E, F]
    maskT_full = m.T.astype(np.uint8)                     # [W, F]
    vT_full = values.T                                    # [F, B]

    in_maps = []
    for c in range(NCORES):
        sl = slice(c * FS, (c + 1) * FS)
        feT = np.zeros((E, FP), np.float32)
        feT[:, :FS] = feT_full[:, sl]
        mT = np.ones((W, FP), np.uint8)                   # pad=1 keeps exp sums finite
        mT[:, :FS] = maskT_full[:, sl]
        # packed [2, PW, 16, FP]: row w = hq*16*PW + i*PW + p -> [hq, p, i, :]
        mT = mT.reshape(2, 16, PW, FP).transpose(0, 2, 1, 3)
        vt = np.zeros((FP, B), np.float32)                # pad=0 kills junk features
        vt[:FS] = vT_full[sl]
        blob = np.zeros((H, 386), np.float32)
        blob[:, 0] = b_w
        blob[:, 1] = W_u[:, 0]
        blob[:E, 2:66] = w1
        blob[:E, 66:130] = w2
        blob[:E, 130:386] = feT
        in_maps.append({
            "blob": blob,
            "heT": heT, "heo": heo,
            "maskT": np.ascontiguousarray(mT),
            "vT": np.ascontiguousarray(vt),
        })
    return in_maps


_CACHED = {}


def kernel(values, feature_emb, hidden_emb, W_w, b_w, W_u, mask):
    _import_concourse()
    from concourse.bass_utils import run_bass_kernel_spmd

    if "nc" not in _CACHED:
        _CACHED["nc"] = build_nc()
    nc = _CACHED["nc"]
    in_maps = shard_inputs(values, feature_emb, hidden_emb, W_w, b_w, W_u, mask)
    res = run_bass_kernel_spmd(nc, in_maps, list(range(NCORES)))
    parts = [res.results[c]["out"] for c in range(NCORES)]
    return np.sum(np.stack(parts, 0), 0, dtype=np.float32)



# revision 23
# speedup vs baseline: 1.0381x; 1.0381x over previous
"""Trainium2 Bass kernel for nn_DescriptionEmbedding (attention-pooling).

Math: for each feature f, attention over W hidden words:
  score[f,w] = sum_h u[h] * tanh(a[f,h] + c[w,h]),  a = fe@W1, c = he@W2 + b
  attn = softmax_w(masked exp), context[f] = sum_w attn*he[w], out = values@context

Reformulation (exact identity + short series, j<=2; validated ~2.5e-3 with
bf16 operands):
  tanh(a+c) = ta + (1-ta^2)tc - (1-ta^2)ta tc^2 + ...
  S[w,f] = tc[w,:] @ P1[f,:].T + tc^2[w,:] @ P2[f,:].T
  P1 = u*(1-ta^2), P2 = -P1*ta   (the j=0 term cancels in softmax)

Per-core layout (F=2000 split 8 x 250, two halves of 125):
 - prep: ONE matmul per 1000-word tile with block-diag [[w2,0],[0,w2]] lhsT
   produces a [128,500] pre-activation; 128-wide tanh (bf16 out) halves ACT
   cost vs 64-wide. DVE squares + Pool copies assemble QT=[tc;tc^2] stacks.
 - score: [128,125]bf16 lhsT x PT[128,250]bf16 -> ps[125,8,250] per tile.
 - exp on ACT (bf16 out), mask multiply on DVE (2x bf16 mode, u8 mask).
 - context: lhsT=eq-chunk [125w,125f], rhs=heo [125w,17] -> ctx[125f,2,17]
   accumulated in PSUM across all 32 chunks: f-on-partition layout gives
   per-partition denominators (col 16) -> no transposes in the epilogue.
 - epilogue: reciprocal + per-partition scale -> values@context partial
   [16,256] per core; host sums the 8 partials.
All pools double-buffered so DMAs/prep of rep r+1 overlap compute of rep r.
"""
import os
import sys

import numpy as np

F, W, E, H, B = 2000, 4000, 16, 64, 256
NCORES = 8
FS = F // NCORES          # 250 features per core
FH = FS // 2              # 125: half-shard (partition dim of ctx)
PW = 125                  # words per chunk
NWC = W // PW             # 32 chunks
NT = 4                    # tiles (1000 words each)
CPT = NWC // NT           # 8 chunks per tile


def _import_concourse():
    # bass2jax executes via jax PJRT on the neuron devices; a cpu platform
    # pin would hide them. Clear it if jax hasn't been imported yet.
    if "jax" not in sys.modules and os.environ.get("JAX_PLATFORMS") == "cpu":
        del os.environ["JAX_PLATFORMS"]
    try:
        import concourse.bass  # noqa: F401
    except ImportError:
        for p in ("/opt/trn_rl_repo", os.path.expanduser("~/trn_rl_repo")):
            if os.path.isdir(p) and p not in sys.path:
                sys.path.insert(0, p)
        import concourse.bass  # noqa: F401


def build_nc(reps=1):
    _import_concourse()
    import concourse.bass as bass  # noqa: F401
    import concourse.mybir as mybir
    import concourse.tile as tile
    from concourse import bacc
    from concourse.alu_op_type import AluOpType

    f32 = mybir.dt.float32
    f32r = mybir.dt.float32r
    bf16 = mybir.dt.bfloat16
    ACT = mybir.ActivationFunctionType

    nc = bacc.Bacc(None, target_bir_lowering=False, debug=False)

    # blob cols (f32r): 0=bb([b;b]), 1=-u, 2=+u, [0:32,3:131]=w2blk,
    # [0:16,131:195]=w1, [0:16,195:445]=feT
    blob = nc.dram_tensor("blob", [128, 445], f32r, kind="ExternalInput")
    heT = nc.dram_tensor("heT", [64, 1000], f32r, kind="ExternalInput")
    heo = nc.dram_tensor("heo", [PW, NWC, 17], bf16, kind="ExternalInput")
    maskT = nc.dram_tensor("maskT", [PW, NWC, 256], bf16, kind="ExternalInput")
    vT = nc.dram_tensor("vT", [FH, 2, B], f32, kind="ExternalInput")
    out = nc.dram_tensor("out", [E, B], f32, kind="ExternalOutput")

    import contextlib

    with tile.TileContext(nc) as tc:
        with tc.tile_pool(name="boot", bufs=1) as boot:
            dummy = boot.tile([1, 2], f32)
            nc.vector.memset(dummy[:], 0)
            nc.scalar.activation(dummy[:], dummy[:], ACT.Exp)
            nc.scalar.activation(dummy[:], dummy[:], ACT.Tanh)
        with (
            tc.tile_pool(name="consts", bufs=2) as consts,
            tc.tile_pool(name="rt", bufs=2) as rpool,
            tc.tile_pool(name="escore", bufs=2) as epool,
            tc.tile_pool(name="small", bufs=2) as small,
            tc.tile_pool(name="hp_ps", bufs=2, space="PSUM") as hp_ps,
            tc.tile_pool(name="s_ps", bufs=1, space="PSUM") as s_ps,
            tc.tile_pool(name="ctx_ps", bufs=1, space="PSUM") as ctx_ps,
        ):
            def epilogue(st):
                # normalize ctx (per-partition denominators) and produce the
                # [E, B] partial via values@context; emitted early in the
                # NEXT body so PE's prep matmuls aren't queued behind it
                ctxh, vTs = st
                rv = small.tile([FH, 2], f32, tag="rv")
                for h in range(2):
                    nc.vector.reciprocal(rv[:, h:h + 1], ctxh[h][:, 16:17])
                ctxn = small.tile([FH, 2, E], f32, tag="ctxn")
                for h in range(2):
                    nc.vector.tensor_scalar_mul(ctxn[:, h, :],
                                                ctxh[h][:, 0:E],
                                                rv[:, h:h + 1])
                po = hp_ps.tile([E, B], f32, tag="hp", name="po")
                for h in range(2):
                    nc.tensor.matmul(po[:], ctxn[:, h, :], vTs[:, h, :],
                                     start=(h == 0), stop=(h == 1))
                outsb = small.tile([E, B], f32, tag="outsb")
                nc.vector.tensor_copy(outsb[:], po[:])
                nc.gpsimd.dma_start(out[:], outsb[:])

            def head():
                """Input DMAs + PT prep + all prep tiles (tanh, QT builds).

                Emitted BEFORE the previous body's main-stage so PE's prep
                matmuls aren't queued behind the previous tail's ctx
                matmuls (which wait on the late mask multiplies).
                """
                # ---- input DMAs (ordered by first use) --------------------
                blobs = consts.tile([128, 445], f32r, name="blobs")
                heTs = consts.tile([64, 1000], f32r, name="heTs")
                mqs = consts.tile([PW, NWC, 256], bf16, name="mqs")
                heos = consts.tile([PW, NWC, 17], bf16, name="heos")
                vTs = consts.tile([FH, 2, B], f32, name="vTs")
                nc.sync.dma_start(blobs[:], blob[:])
                nc.sync.dma_start(heTs[:], heT[:])
                nc.sync.dma_start(mqs[:], maskT[:])
                nc.sync.dma_start(heos[:], heo[:])
                nc.sync.dma_start(vTs[:], vT[:])

                bbs = blobs[:, 0:1].bitcast(f32)
                nus = blobs[0:H, 1:2].bitcast(f32)   # -u
                pus = blobs[0:H, 2:3].bitcast(f32)   # +u
                w1s = blobs[0:E, 131:195]
                feTs = blobs[0:E, 195:445]

                # ---- PT prep: PT[0:64]=u(1-ta^2), PT[64:128]=-PT1*ta ------
                pf = hp_ps.tile([H, FS], f32, tag="hp", name="pf")
                nc.tensor.matmul(pf[:], w1s, feTs, start=True, stop=True)
                ta = small.tile([H, FS], bf16, tag="ta")
                nc.scalar.activation(ta[:], pf[:], ACT.Tanh)
                PT = consts.tile([128, 256], bf16, name="PT")
                nc.vector.memset(PT[:, FS:256], 0)
                sq = small.tile([H, FS], bf16, tag="sq")
                nc.vector.tensor_tensor(sq[:], ta[:], ta[:], AluOpType.mult)
                # PT1 = (sq * -u) + u
                nc.vector.tensor_scalar(PT[0:H, 0:FS], sq[:], nus, pus,
                                        AluOpType.mult, AluOpType.add)
                # PT2 = (ta * -1) * PT1
                nc.vector.scalar_tensor_tensor(PT[H:128, 0:FS], ta[:], -1.0,
                                               PT[0:H, 0:FS], AluOpType.mult,
                                               AluOpType.mult)

                ctxh = [ctx_ps.tile([FH, 17], f32, name=f"ctx{h}")
                        for h in range(2)]
                QTs = [consts.tile([128, 2 * 500], bf16, name=f"QT{t}")
                       for t in range(NT)]

                for t in range(NT):
                    # hp[0:64] = w2.T @ heT(words 1000t..+500); hp[64:128] =
                    # words +500..+1000 via block-diag w2blk (bias in tanh)
                    hp = hp_ps.tile([128, 500], f32, tag="hp", name=f"hp{t}")
                    bnd = 32 * (t // 2)
                    nc.tensor.matmul(hp[:], blobs[bnd:bnd + 32, 3:131],
                                     heTs[bnd:bnd + 32,
                                          500 * (t % 2):500 * (t % 2) + 500],
                                     start=True, stop=True)
                    r = rpool.tile([128, 500], bf16, tag="r", name=f"r{t}")
                    nc.scalar.activation(r[:], hp[:], ACT.Tanh, bias=bbs)
                    qt = QTs[t]
                    # tc rows: copies (Pool); tc^2 rows: squares (DVE)
                    nc.gpsimd.tensor_copy(qt[0:H, 0:500], r[0:H, :])
                    nc.gpsimd.tensor_copy(qt[0:H, 500:1000], r[H:128, :])
                    nc.vector.tensor_tensor(qt[H:128, 0:500], r[0:H, :],
                                            r[0:H, :], AluOpType.mult)
                    nc.vector.tensor_tensor(qt[H:128, 500:1000], r[H:128, :],
                                            r[H:128, :], AluOpType.mult)
                return (ctxh, QTs, PT, mqs, heos, vTs)

            def main_stage(st):
                """Scores, exps, mask multiplies, ctx accumulation, epilogue
                for the body whose head() produced ``st``."""
                ctxh, QTs, PT, mqs, heos, vTs = st
                for t in range(NT):
                    # two 2-bank ps tiles so next tile's score matmuls can
                    # start as soon as the matching exp half has drained
                    psh = [s_ps.tile([PW, CPT // 2, 256], f32, tag=f"ps{g}",
                                     name=f"ps{t}_{g}") for g in range(2)]
                    for i in range(CPT):
                        nc.tensor.matmul(psh[i // 4][:, i % 4, :],
                                         QTs[t][:, PW * i:PW * i + PW],
                                         PT[:], start=True, stop=True)
                    eq = epool.tile([PW, CPT, 256], bf16, name=f"eq{t}")
                    for g in range(2):
                        nc.scalar.activation(eq[:, 4 * g:4 * g + 4, :],
                                             psh[g][:], ACT.Exp)
                    nc.vector.tensor_tensor(eq[:], eq[:],
                                            mqs[:, CPT * t:CPT * t + CPT, :],
                                            AluOpType.mult)
                    for i in range(CPT):
                        wc = CPT * t + i
                        for h in range(2):
                            nc.tensor.matmul(ctxh[h][:],
                                             eq[:, i, FH * h:FH * h + FH],
                                             heos[:, wc, :],
                                             start=(wc == 0),
                                             stop=(wc == NWC - 1))
                epilogue((ctxh, vTs))

            # Manual unroll: U bodies per For_i iteration so the loop's
            # all-engine barrier amortizes; two-stage software pipeline
            # (head of body k emitted before main of body k-1) keeps every
            # engine queue free of cross-body blocking.
            U = 8 if reps >= 8 else 1
            n_groups, rem = divmod(reps, U)

            def run_chain(n):
                st = head()
                for _ in range(n - 1):
                    st_next = head()
                    main_stage(st)
                    st = st_next
                main_stage(st)

            if n_groups >= 1:
                with tc.For_i(0, n_groups, 1):
                    run_chain(U)
            if rem:
                run_chain(rem)

    nc.compile()
    return nc


def shard_inputs(values, feature_emb, hidden_emb, W_w, b_w, W_u, mask):
    """Host-side shard/layout prep. Returns per-core input maps."""
    from ml_dtypes import bfloat16

    values = np.ascontiguousarray(values, dtype=np.float32)
    fe = np.ascontiguousarray(feature_emb, dtype=np.float32)
    he = np.ascontiguousarray(hidden_emb, dtype=np.float32)
    W_w = np.ascontiguousarray(W_w, dtype=np.float32)
    b_w = np.ascontiguousarray(b_w, dtype=np.float32)
    W_u = np.ascontiguousarray(W_u, dtype=np.float32)
    m = np.asarray(mask).reshape(F, W)

    w1 = W_w[:E]                                          # [E, H]
    w2 = W_w[E:]                                          # [E, H]
    w2blk = np.zeros((32, 128), np.float32)
    w2blk[0:E, 0:H] = w2
    w2blk[E:32, H:128] = w2

    heT_full = he.T                                       # [E, W]
    # [64, 1000]: tile t at rows 32*(t//2), cols 500*(t%2); each tile's two
    # 500-word halves stacked 16+16 on the partition dim (block-diag w2blk)
    heT = np.zeros((64, 1000), np.float32)
    for t in range(NT):
        r0, c0 = 32 * (t // 2), 500 * (t % 2)
        heT[r0:r0 + 16, c0:c0 + 500] = heT_full[:, 1000 * t:1000 * t + 500]
        heT[r0 + 16:r0 + 32, c0:c0 + 500] = \
            heT_full[:, 1000 * t + 500:1000 * t + 1000]

    heo_flat = np.concatenate([he, np.ones((W, 1), np.float32)], axis=1)
    heo = np.ascontiguousarray(
        heo_flat.reshape(NWC, PW, 17).transpose(1, 0, 2)).astype(bfloat16)

    mT_full = m.T.astype(bfloat16)                        # [W, F]
    vT_full = values.T                                    # [F, B]
    feT_full = fe.T                                       # [E, F]

    in_maps = []
    for c in range(NCORES):
        sl = slice(c * FS, (c + 1) * FS)
        blob = np.zeros((128, 445), np.float32)
        blob[:, 0] = np.concatenate([b_w, b_w])
        blob[0:H, 1] = -W_u[:, 0]
        blob[0:H, 2] = W_u[:, 0]
        blob[0:64, 3:131] = np.tile(w2blk, (2, 1))
        blob[0:E, 131:195] = w1
        blob[0:E, 195:445] = feT_full[:, sl]
        mq = np.zeros((PW, NWC, 256), bfloat16)
        mq[:, :, :FS] = mT_full[:, sl].reshape(NWC, PW, FS).transpose(1, 0, 2)
        vt = np.ascontiguousarray(
            vT_full[sl].reshape(2, FH, B).transpose(1, 0, 2))
        in_maps.append({
            "blob": blob,
            "heT": heT, "heo": heo,
            "maskT": mq,
            "vT": vt,
        })
    return in_maps


_CACHED = {}


def kernel(values, feature_emb, hidden_emb, W_w, b_w, W_u, mask):
    _import_concourse()
    from concourse.bass_utils import run_bass_kernel_spmd

    if "nc" not in _CACHED:
        _CACHED["nc"] = build_nc()
    nc = _CACHED["nc"]
    in_maps = shard_inputs(values, feature_emb, hidden_emb, W_w, b_w, W_u, mask)
    res = run_bass_kernel_spmd(nc, in_maps, list(range(NCORES)))
    parts = [res.results[c]["out"] for c in range(NCORES)]
    acc = np.sum(np.stack(parts, 0), 0, dtype=np.float32)   # [E, B]
    return np.ascontiguousarray(acc.T)
